# revision 4
# baseline (speedup 1.0000x reference)
"""Trainium2 Bass kernel for nn_CollatedVanillaCNN.

The model applies a tiny CNN (log1p -> conv3x3(16->32)+bn+relu+avgpool2 ->
conv3x3(32->64)+bn+relu+avgpool2 -> fc(64->16)+bn+relu -> fc(16->8) -> expm1)
independently to the 4x4 sliding window at every pixel of x[4,16,128,128]
(zero-padded right/bottom), producing out[4,8,128,128].

Strategy: every output pixel is an independent sample => express the whole
network as 4 dense matmul stages over pixels (features on SBUF partitions,
pixels on the free dim):

  conv1 : windows  K=(sh,sw,c)=256  ->  M=(pw,qw,o1)=512   (masked 3x3 taps)
  conv2 : K=(pw,qw,o1)=512 -> M=(r,t,o2)=256               (avgpool1 folded in)
  fc1   : K=(r,t,o2)=256   -> M=16                         (avgpool2 folded in)
  fc2   : K=16 -> M=8

bn scales are folded into the weight columns; bn/conv biases are applied via
per-partition bias operands of the scalar-engine activation (relu / exp) or
vector-engine tensor_scalar.  Matmuls run as float32r (full-rate fp32).

Sharding: pure data parallel over B x H/2: core = (b, row half), 8192 pixels
per core, tiled as 16 tiles of 512 pixels (4 image rows).  Host does only
data movement (pad/im2col/layout); all arithmetic runs on device.
"""

import numpy as np

import concourse.bacc as bacc
import concourse.bass as bass
import concourse.mybir as mybir
import concourse.tile as tile
from concourse import bass_utils

AF = mybir.ActivationFunctionType
ALU = mybir.AluOpType
F32 = mybir.dt.float32
F32R = mybir.dt.float32r

EPS = 1e-5
NCORES = 8
NT = 16          # pixel tiles per core (each 4 image rows x 128 cols = 512 px)


# ---------------------------------------------------------------- host packing

def _pack_weights(p):
    """Pack all network params into device-layout matmul weights / biases."""
    w1 = p["conv1_w"].astype(np.float64)   # [32,16,3,3]
    w2 = p["conv2_w"].astype(np.float64)   # [64,32,3,3]
    s1 = (p["bn1_g"] / np.sqrt(p["bn1_v"] + EPS)).astype(np.float64)
    s2 = (p["bn2_g"] / np.sqrt(p["bn2_v"] + EPS)).astype(np.float64)
    s3 = (p["bn3_g"] / np.sqrt(p["bn3_v"] + EPS)).astype(np.float64)

    # conv1: rows (h,p) -> (sh,c,sw), cols f=(pw*4+qw)*32+o
    W1 = np.zeros((2, 128, 512), np.float64)
    pp = np.arange(128)
    c_of_p = (pp % 64) // 4
    sw_of_p = pp % 4
    f = np.arange(512)
    pw_of_f = f // 128
    qw_of_f = (f % 128) // 32
    o_of_f = f % 32
    for h in range(2):
        sh = 2 * h + pp // 64                       # [128]
        du = sh[:, None] - pw_of_f[None, :] + 1      # [128,512]
        dv = sw_of_p[:, None] - qw_of_f[None, :] + 1
        valid = (du >= 0) & (du < 3) & (dv >= 0) & (dv < 3)
        duc = np.clip(du, 0, 2)
        dvc = np.clip(dv, 0, 2)
        vals = w1[o_of_f[None, :].repeat(128, 0),
                  c_of_p[:, None].repeat(512, 1),
                  duc, dvc]
        W1[h] = np.where(valid, vals, 0.0) * s1[o_of_f][None, :]
    W1 = W1.transpose(1, 0, 2).reshape(128, 1024)    # [k, h*512+f]
    bias1 = ((p["conv1_b"] - p["bn1_m"]) * s1 + p["bn1_b"])  # [32] by o
    b1 = np.tile(bias1, 4).reshape(128, 1)           # partition (qw*32+o)

    # conv2 (+avgpool1): chunk pw, rows (qw*32+o1), cols f2=(r*2+t)*64+o2
    W2 = np.zeros((4, 128, 256), np.float64)
    row = np.arange(128)
    q_of_row = row // 32
    o1_of_row = row % 32
    f2 = np.arange(256)
    r_of_f2 = f2 // 128
    t_of_f2 = (f2 % 128) // 64
    o2_of_f2 = f2 % 64
    for pw in range(4):
        du = (pw // 2) - r_of_f2 + 1                 # [256] in {0,1,2}
        dv = (q_of_row // 2)[:, None] - t_of_f2[None, :] + 1
        vals = w2[o2_of_f2[None, :].repeat(128, 0),
                  o1_of_row[:, None].repeat(256, 1),
                  du[None, :].repeat(128, 0), dv]
        W2[pw] = 0.25 * vals * s2[o2_of_f2][None, :]
    W2 = W2.transpose(1, 0, 2).reshape(128, 1024)    # [row, pw*256+f2]
    bias2 = ((p["conv2_b"] - p["bn2_m"]) * s2 + p["bn2_b"])  # [64] by o2
    b2 = np.tile(bias2, 2).reshape(128, 1)           # partition (t*64+o2)

    # fc1 (+avgpool2): chunk r, rows (t*64+o2), cols m
    # rows (t,o2): value 0.25*fc1_w[m, o2]*s3[m], independent of (r,t)
    base = 0.25 * p["fc1_w"].astype(np.float64).T * s3[None, :]  # [64,16]
    W3 = np.stack([np.tile(base, (2, 1))] * 2)       # [2,128,16]
    W3 = W3.transpose(1, 0, 2).reshape(128, 32)      # [row, r*16+m]
    b3 = (((p["fc1_b"] - p["bn3_m"]) * s3 + p["bn3_b"]).reshape(16, 1))

    # fc2: rows m, cols o
    W4 = p["fc2_w"].astype(np.float64).T             # [16,8]
    b4 = p["fc2_b"].reshape(8, 1)

    return {
        "w1": np.ascontiguousarray(W1, np.float32),
        "w2": np.ascontiguousarray(W2, np.float32),
        "w3": np.ascontiguousarray(W3, np.float32),
        "w4": np.ascontiguousarray(W4, np.float32),
        "b1": np.ascontiguousarray(b1, np.float32),
        "b2": np.ascontiguousarray(b2, np.float32),
        "b3": np.ascontiguousarray(b3, np.float32),
        "b4": np.ascontiguousarray(b4, np.float32),
    }


def _im2col_core(xs):
    """xs: [16, 67, 131] padded row-slab -> xcols [16 tiles, 128, 1024]."""
    xcols = np.empty((NT, 128, 1024), np.float32)
    for sh in range(4):
        h, lo = divmod(sh, 2)
        for sw in range(4):
            blk = xs[:, sh:sh + 64, sw:sw + 128]         # [16c, 64, 128]
            blk = blk.reshape(16, NT, 4 * 128)           # [c, t, px]
            parts = 64 * lo + np.arange(16) * 4 + sw     # dest partitions
            xcols[:, parts, 512 * h:512 * h + 512] = blk.transpose(1, 0, 2)
    return xcols


def _make_in_maps(inputs):
    x = np.asarray(inputs["x"], np.float32)              # [4,16,128,128]
    xp = np.pad(x, ((0, 0), (0, 0), (0, 3), (0, 3)))     # [4,16,131,131]
    packed = _pack_weights({k: np.asarray(v, np.float64) for k, v in inputs.items()
                            if k != "x"})
    in_maps = []
    for core in range(NCORES):
        b, half = divmod(core, 2)
        r0 = half * 64
        xs = xp[b, :, r0:r0 + 67, :]
        m = dict(packed)
        m["xcols"] = _im2col_core(xs)
        in_maps.append(m)
    return in_maps


# ---------------------------------------------------------------- device build

def build_nc():
    nc = bacc.Bacc("TRN2", target_bir_lowering=False, debug=False,
                   num_devices=NCORES)
    xcols_d = nc.dram_tensor("xcols", [NT, 128, 1024], F32, kind="ExternalInput")
    w1_d = nc.dram_tensor("w1", [128, 1024], F32R, kind="ExternalInput")
    w2_d = nc.dram_tensor("w2", [128, 1024], F32R, kind="ExternalInput")
    w3_d = nc.dram_tensor("w3", [128, 32], F32R, kind="ExternalInput")
    w4_d = nc.dram_tensor("w4", [16, 8], F32R, kind="ExternalInput")
    b1_d = nc.dram_tensor("b1", [128, 1], F32, kind="ExternalInput")
    b2_d = nc.dram_tensor("b2", [128, 1], F32, kind="ExternalInput")
    b3_d = nc.dram_tensor("b3", [16, 1], F32, kind="ExternalInput")
    b4_d = nc.dram_tensor("b4", [8, 1], F32, kind="ExternalInput")
    out_d = nc.dram_tensor("out", [8, 64, 128], F32, kind="ExternalOutput")

    with tile.TileContext(nc) as tc:
        with (
            tc.tile_pool(name="wpool", bufs=1) as wpool,
            tc.tile_pool(name="xin", bufs=3) as xin,
            tc.tile_pool(name="rhs1", bufs=2) as rhs1p,
            tc.tile_pool(name="relu1", bufs=2) as relu1p,
            tc.tile_pool(name="relu2", bufs=2) as relu2p,
            tc.tile_pool(name="relu3", bufs=2) as relu3p,
            tc.tile_pool(name="outsb", bufs=2) as outp,
            tc.tile_pool(name="ps1", bufs=1, space="PSUM") as ps1,
            tc.tile_pool(name="ps2", bufs=1, space="PSUM") as ps2,
            tc.tile_pool(name="psf1", bufs=1, space="PSUM") as psf1,
            tc.tile_pool(name="psf2", bufs=1, space="PSUM") as psf2,
        ):
            w1s = wpool.tile([128, 1024], F32R)
            w2s = wpool.tile([128, 1024], F32R)
            w3s = wpool.tile([128, 32], F32R)
            w4s = wpool.tile([16, 8], F32R)
            b1s = wpool.tile([128, 1], F32)
            b2s = wpool.tile([128, 1], F32)
            b3s = wpool.tile([16, 1], F32)
            b4s = wpool.tile([8, 1], F32)
            nc.sync.dma_start(w1s[:], w1_d[:])
            nc.sync.dma_start(w2s[:], w2_d[:])
            nc.sync.dma_start(w3s[:], w3_d[:])
            nc.sync.dma_start(w4s[:], w4_d[:])
            nc.sync.dma_start(b1s[:], b1_d[:])
            nc.sync.dma_start(b2s[:], b2_d[:])
            nc.sync.dma_start(b3s[:], b3_d[:])
            nc.sync.dma_start(b4s[:], b4_d[:])

            for t in range(NT):
                xt = xin.tile([128, 1024], F32)
                nc.sync.dma_start(xt[:], xcols_d[t])

                # log1p
                r1 = rhs1p.tile([128, 1024], F32R)
                nc.scalar.activation(r1[:], xt[:], AF.Ln, bias=1.0, scale=1.0)

                # conv1 (+bn1 scale): K=256 (2 chunks), M=512 (4 chunks)
                o1 = ps1.tile([128, 2048], F32)
                for m in range(4):
                    for h in range(2):
                        nc.tensor.matmul(
                            o1[:, 512 * m:512 * (m + 1)],
                            w1s[:, 512 * h + 128 * m:512 * h + 128 * m + 128],
                            r1[:, 512 * h:512 * h + 512],
                            start=(h == 0), stop=(h == 1),
                        )
                # bn1 bias + relu
                rl1 = relu1p.tile([128, 2048], F32R)
                nc.scalar.activation(rl1[:], o1[:], AF.Relu, bias=b1s[:],
                                     scale=1.0)

                # conv2 (+pool1, +bn2 scale): K=512 (4 chunks), M=256 (2 chunks)
                o2 = ps2.tile([128, 1024], F32)
                for n in range(2):
                    for pw in range(4):
                        nc.tensor.matmul(
                            o2[:, 512 * n:512 * (n + 1)],
                            w2s[:, 256 * pw + 128 * n:256 * pw + 128 * n + 128],
                            rl1[:, 512 * pw:512 * pw + 512],
                            start=(pw == 0), stop=(pw == 3),
                        )
                # bn2 bias + relu (vector engine)
                rl2 = relu2p.tile([128, 1024], F32R)
                nc.vector.tensor_scalar(rl2[:], o2[:], b2s[:], 0.0,
                                        op0=ALU.add, op1=ALU.max)

                # fc1 (+pool2, +bn3 scale): K=256 (2 chunks), M=16
                f1 = psf1.tile([16, 512], F32)
                for h in range(2):
                    nc.tensor.matmul(
                        f1[:],
                        w3s[:, 16 * h:16 * h + 16],
                        rl2[:, 512 * h:512 * h + 512],
                        start=(h == 0), stop=(h == 1),
                    )
                rl3 = relu3p.tile([16, 512], F32R)
                nc.vector.tensor_scalar(rl3[:], f1[:], b3s[:], 0.0,
                                        op0=ALU.add, op1=ALU.max)

                # fc2: K=16, M=8
                f2 = psf2.tile([8, 512], F32)
                nc.tensor.matmul(f2[:], w4s[:], rl3[:],
                                 start=True, stop=True)

                # expm1 = exp(x + fc2_b) - 1
                ob = outp.tile([8, 512], F32, tag="ob")
                nc.scalar.activation(ob[:], f2[:], AF.Exp, bias=b4s[:],
                                     scale=1.0)
                ob2 = outp.tile([8, 512], F32, tag="ob2")
                nc.vector.tensor_scalar(ob2[:], ob[:], 1.0, None,
                                        op0=ALU.subtract)

                nc.sync.dma_start(out_d[:, 4 * t:4 * t + 4, :], ob2[:])

    nc.compile()
    return nc


_NC = None


def _get_nc():
    global _NC
    if _NC is None:
        _NC = build_nc()
    return _NC


def _assemble(results):
    out = np.empty((4, 8, 128, 128), np.float32)
    for core in range(NCORES):
        b, half = divmod(core, 2)
        out[b, :, half * 64:half * 64 + 64, :] = results[core]["out"]
    return out


def kernel(_trace=False, **inputs):
    nc = _get_nc()
    in_maps = _make_in_maps(inputs)
    res = bass_utils.run_bass_kernel_spmd(
        nc, in_maps, core_ids=list(range(NCORES)), trace=_trace)
    out = _assemble(res.results)
    if _trace:
        return out, res
    return out


def kernel_sim(**inputs):
    """CoreSim-based check of a single core (core 0) against its slice."""
    from concourse.bass_interp import CoreSim
    nc = _get_nc()
    in_maps = _make_in_maps(inputs)
    outs = []
    for core in range(NCORES):
        sim = CoreSim(nc, trace=False, require_finite=False, require_nnan=False)
        for k, v in in_maps[core].items():
            sim.tensor(k)[:] = v
        sim.simulate()
        outs.append({"out": sim.tensor("out").copy()})
    return _assemble(outs)


# revision 5
# speedup vs baseline: 1.0100x; 1.0100x over previous
"""Trainium2 Bass kernel for nn_CollatedVanillaCNN.

The model applies a tiny CNN (log1p -> conv3x3(16->32)+bn+relu+avgpool2 ->
conv3x3(32->64)+bn+relu+avgpool2 -> fc(64->16)+bn+relu -> fc(16->8) -> expm1)
independently to the 4x4 sliding window at every pixel of x[4,16,128,128]
(zero-padded right/bottom), producing out[4,8,128,128].

Strategy: every output pixel is an independent sample => express the whole
network as 4 dense matmul stages over pixels (features on SBUF partitions,
pixels on the free dim):

  conv1 : windows  K=(sh,sw,c)=256  ->  M=(pw,qw,o1)=512   (masked 3x3 taps)
  conv2 : K=(pw,qw,o1)=512 -> M=(r,t,o2)=256               (avgpool1 folded in)
  fc1   : K=(r,t,o2)=256   -> M=16                         (avgpool2 folded in)
  fc2   : K=16 -> M=8

bn scales are folded into the weight columns; bn/conv biases are applied via
per-partition bias operands of the scalar-engine activation (relu / exp) or
vector-engine tensor_scalar.  Matmuls run as float32r (full-rate fp32).

Sharding: pure data parallel over B x H/2: core = (b, row half), 8192 pixels
per core, tiled as 16 tiles of 512 pixels (4 image rows).  Host does only
data movement (pad/im2col/layout); all arithmetic runs on device.
"""

import numpy as np

import concourse.bacc as bacc
import concourse.bass as bass
import concourse.mybir as mybir
import concourse.tile as tile
from concourse import bass_utils

AF = mybir.ActivationFunctionType
ALU = mybir.AluOpType
F32 = mybir.dt.float32
F32R = mybir.dt.float32r

EPS = 1e-5
NCORES = 8
NT = 16          # pixel tiles per core (each 4 image rows x 128 cols = 512 px)


# ---------------------------------------------------------------- host packing

def _pack_weights(p):
    """Pack all network params into device-layout matmul weights / biases."""
    w1 = p["conv1_w"].astype(np.float64)   # [32,16,3,3]
    w2 = p["conv2_w"].astype(np.float64)   # [64,32,3,3]
    s1 = (p["bn1_g"] / np.sqrt(p["bn1_v"] + EPS)).astype(np.float64)
    s2 = (p["bn2_g"] / np.sqrt(p["bn2_v"] + EPS)).astype(np.float64)
    s3 = (p["bn3_g"] / np.sqrt(p["bn3_v"] + EPS)).astype(np.float64)

    # conv1: rows (h,p) -> (sh,c,sw), cols f=(pw*4+qw)*32+o
    W1 = np.zeros((2, 128, 512), np.float64)
    pp = np.arange(128)
    c_of_p = (pp % 64) // 4
    sw_of_p = pp % 4
    f = np.arange(512)
    pw_of_f = f // 128
    qw_of_f = (f % 128) // 32
    o_of_f = f % 32
    for h in range(2):
        sh = 2 * h + pp // 64                       # [128]
        du = sh[:, None] - pw_of_f[None, :] + 1      # [128,512]
        dv = sw_of_p[:, None] - qw_of_f[None, :] + 1
        valid = (du >= 0) & (du < 3) & (dv >= 0) & (dv < 3)
        duc = np.clip(du, 0, 2)
        dvc = np.clip(dv, 0, 2)
        vals = w1[o_of_f[None, :].repeat(128, 0),
                  c_of_p[:, None].repeat(512, 1),
                  duc, dvc]
        W1[h] = np.where(valid, vals, 0.0) * s1[o_of_f][None, :]
    W1 = W1.transpose(1, 0, 2).reshape(128, 1024)    # [k, h*512+f]
    bias1 = ((p["conv1_b"] - p["bn1_m"]) * s1 + p["bn1_b"])  # [32] by o
    b1 = np.tile(bias1, 4).reshape(128, 1)           # partition (qw*32+o)

    # conv2 (+avgpool1): chunk pw, rows (qw*32+o1), cols f2=(r*2+t)*64+o2
    W2 = np.zeros((4, 128, 256), np.float64)
    row = np.arange(128)
    q_of_row = row // 32
    o1_of_row = row % 32
    f2 = np.arange(256)
    r_of_f2 = f2 // 128
    t_of_f2 = (f2 % 128) // 64
    o2_of_f2 = f2 % 64
    for pw in range(4):
        du = (pw // 2) - r_of_f2 + 1                 # [256] in {0,1,2}
        dv = (q_of_row // 2)[:, None] - t_of_f2[None, :] + 1
        vals = w2[o2_of_f2[None, :].repeat(128, 0),
                  o1_of_row[:, None].repeat(256, 1),
                  du[None, :].repeat(128, 0), dv]
        W2[pw] = 0.25 * vals * s2[o2_of_f2][None, :]
    W2 = W2.transpose(1, 0, 2).reshape(128, 1024)    # [row, pw*256+f2]
    bias2 = ((p["conv2_b"] - p["bn2_m"]) * s2 + p["bn2_b"])  # [64] by o2
    b2 = np.tile(bias2, 2).reshape(128, 1)           # partition (t*64+o2)

    # fc1 (+avgpool2): chunk r, rows (t*64+o2), cols m
    # rows (t,o2): value 0.25*fc1_w[m, o2]*s3[m], independent of (r,t)
    base = 0.25 * p["fc1_w"].astype(np.float64).T * s3[None, :]  # [64,16]
    W3 = np.stack([np.tile(base, (2, 1))] * 2)       # [2,128,16]
    W3 = W3.transpose(1, 0, 2).reshape(128, 32)      # [row, r*16+m]
    b3 = (((p["fc1_b"] - p["bn3_m"]) * s3 + p["bn3_b"]).reshape(16, 1))

    # fc2: rows m, cols o
    W4 = p["fc2_w"].astype(np.float64).T             # [16,8]
    b4 = p["fc2_b"].reshape(8, 1)

    return {
        "w1": np.ascontiguousarray(W1, np.float32),
        "w2": np.ascontiguousarray(W2, np.float32),
        "w3": np.ascontiguousarray(W3, np.float32),
        "w4": np.ascontiguousarray(W4, np.float32),
        "b1": np.ascontiguousarray(b1, np.float32),
        "b2": np.ascontiguousarray(b2, np.float32),
        "b3": np.ascontiguousarray(b3, np.float32),
        "b4": np.ascontiguousarray(b4, np.float32),
    }


def _im2col_core(xs):
    """xs: [16, 67, 131] padded row-slab -> xcols [16 tiles, 128, 1024]."""
    xcols = np.empty((NT, 128, 1024), np.float32)
    for sh in range(4):
        h, lo = divmod(sh, 2)
        for sw in range(4):
            blk = xs[:, sh:sh + 64, sw:sw + 128]         # [16c, 64, 128]
            blk = blk.reshape(16, NT, 4 * 128)           # [c, t, px]
            parts = 64 * lo + np.arange(16) * 4 + sw     # dest partitions
            xcols[:, parts, 512 * h:512 * h + 512] = blk.transpose(1, 0, 2)
    return xcols


def _make_in_maps(inputs):
    x = np.asarray(inputs["x"], np.float32)              # [4,16,128,128]
    xp = np.pad(x, ((0, 0), (0, 0), (0, 3), (0, 3)))     # [4,16,131,131]
    packed = _pack_weights({k: np.asarray(v, np.float64) for k, v in inputs.items()
                            if k != "x"})
    in_maps = []
    for core in range(NCORES):
        b, half = divmod(core, 2)
        r0 = half * 64
        xs = xp[b, :, r0:r0 + 67, :]
        m = dict(packed)
        m["xcols"] = _im2col_core(xs)
        in_maps.append(m)
    return in_maps


# ---------------------------------------------------------------- device build

def _pin_act_table_set():
    """Force every activation onto natural_log_exp_and_others (has Ln, Exp,
    Relu, Copy): the default per-function greedy set choice alternates table
    sets across Ln/Relu/Exp and burns ~2.7us per ACT_TABLE_LOAD, 28x."""
    from concourse.hw_specs import get_activation_tables as orig
    keep = "natural_log_exp_and_others"

    def patched(arch):
        t = orig(arch)
        return {name: (funcs if name == keep else set())
                for name, funcs in t.items()}

    bacc.get_activation_tables = patched


def build_nc():
    _pin_act_table_set()
    nc = bacc.Bacc("TRN2", target_bir_lowering=False, debug=False,
                   num_devices=NCORES)
    xcols_d = nc.dram_tensor("xcols", [NT, 128, 1024], F32, kind="ExternalInput")
    w1_d = nc.dram_tensor("w1", [128, 1024], F32R, kind="ExternalInput")
    w2_d = nc.dram_tensor("w2", [128, 1024], F32R, kind="ExternalInput")
    w3_d = nc.dram_tensor("w3", [128, 32], F32R, kind="ExternalInput")
    w4_d = nc.dram_tensor("w4", [16, 8], F32R, kind="ExternalInput")
    b1_d = nc.dram_tensor("b1", [128, 1], F32, kind="ExternalInput")
    b2_d = nc.dram_tensor("b2", [128, 1], F32, kind="ExternalInput")
    b3_d = nc.dram_tensor("b3", [16, 1], F32, kind="ExternalInput")
    b4_d = nc.dram_tensor("b4", [8, 1], F32, kind="ExternalInput")
    out_d = nc.dram_tensor("out", [8, 64, 128], F32, kind="ExternalOutput")

    with tile.TileContext(nc) as tc:
        with (
            tc.tile_pool(name="wpool", bufs=1) as wpool,
            tc.tile_pool(name="xin", bufs=3) as xin,
            tc.tile_pool(name="rhs1", bufs=2) as rhs1p,
            tc.tile_pool(name="relu1", bufs=2) as relu1p,
            tc.tile_pool(name="relu2", bufs=2) as relu2p,
            tc.tile_pool(name="relu3", bufs=2) as relu3p,
            tc.tile_pool(name="outsb", bufs=2) as outp,
            tc.tile_pool(name="ps1", bufs=1, space="PSUM") as ps1,
            tc.tile_pool(name="ps2", bufs=1, space="PSUM") as ps2,
            tc.tile_pool(name="psf1", bufs=1, space="PSUM") as psf1,
            tc.tile_pool(name="psf2", bufs=1, space="PSUM") as psf2,
        ):
            w1s = wpool.tile([128, 1024], F32R)
            w2s = wpool.tile([128, 1024], F32R)
            w3s = wpool.tile([128, 32], F32R)
            w4s = wpool.tile([16, 8], F32R)
            b1s = wpool.tile([128, 1], F32)
            b2s = wpool.tile([128, 1], F32)
            b3s = wpool.tile([16, 1], F32)
            b4s = wpool.tile([8, 1], F32)
            nc.sync.dma_start(w1s[:], w1_d[:])
            nc.sync.dma_start(w2s[:], w2_d[:])
            nc.sync.dma_start(w3s[:], w3_d[:])
            nc.sync.dma_start(w4s[:], w4_d[:])
            nc.sync.dma_start(b1s[:], b1_d[:])
            nc.sync.dma_start(b2s[:], b2_d[:])
            nc.sync.dma_start(b3s[:], b3_d[:])
            nc.sync.dma_start(b4s[:], b4_d[:])

            for t in range(NT):
                xt = xin.tile([128, 1024], F32)
                nc.sync.dma_start(xt[:], xcols_d[t])

                # log1p
                r1 = rhs1p.tile([128, 1024], F32R)
                nc.scalar.activation(r1[:], xt[:], AF.Ln, bias=1.0, scale=1.0)

                # conv1 (+bn1 scale): K=256 (2 chunks), M=512 (4 chunks)
                o1 = ps1.tile([128, 2048], F32)
                for m in range(4):
                    for h in range(2):
                        nc.tensor.matmul(
                            o1[:, 512 * m:512 * (m + 1)],
                            w1s[:, 512 * h + 128 * m:512 * h + 128 * m + 128],
                            r1[:, 512 * h:512 * h + 512],
                            start=(h == 0), stop=(h == 1),
                        )
                # bn1 bias + relu
                rl1 = relu1p.tile([128, 2048], F32R)
                nc.scalar.activation(rl1[:], o1[:], AF.Relu, bias=b1s[:],
                                     scale=1.0)

                # conv2 (+pool1, +bn2 scale): K=512 (4 chunks), M=256 (2 chunks)
                o2 = ps2.tile([128, 1024], F32)
                for n in range(2):
                    for pw in range(4):
                        nc.tensor.matmul(
                            o2[:, 512 * n:512 * (n + 1)],
                            w2s[:, 256 * pw + 128 * n:256 * pw + 128 * n + 128],
                            rl1[:, 512 * pw:512 * pw + 512],
                            start=(pw == 0), stop=(pw == 3),
                        )
                # bn2 bias + relu (vector engine)
                rl2 = relu2p.tile([128, 1024], F32R)
                nc.vector.tensor_scalar(rl2[:], o2[:], b2s[:], 0.0,
                                        op0=ALU.add, op1=ALU.max)

                # fc1 (+pool2, +bn3 scale): K=256 (2 chunks), M=16
                f1 = psf1.tile([16, 512], F32)
                for h in range(2):
                    nc.tensor.matmul(
                        f1[:],
                        w3s[:, 16 * h:16 * h + 16],
                        rl2[:, 512 * h:512 * h + 512],
                        start=(h == 0), stop=(h == 1),
                    )
                rl3 = relu3p.tile([16, 512], F32R)
                nc.vector.tensor_scalar(rl3[:], f1[:], b3s[:], 0.0,
                                        op0=ALU.add, op1=ALU.max)

                # fc2: K=16, M=8
                f2 = psf2.tile([8, 512], F32)
                nc.tensor.matmul(f2[:], w4s[:], rl3[:],
                                 start=True, stop=True)

                # expm1 = exp(x + fc2_b) - 1
                ob = outp.tile([8, 512], F32, tag="ob")
                nc.scalar.activation(ob[:], f2[:], AF.Exp, bias=b4s[:],
                                     scale=1.0)
                ob2 = outp.tile([8, 512], F32, tag="ob2")
                nc.vector.tensor_scalar(ob2[:], ob[:], 1.0, None,
                                        op0=ALU.subtract)

                nc.sync.dma_start(out_d[:, 4 * t:4 * t + 4, :], ob2[:])

    nc.compile()
    return nc


_NC = None


def _get_nc():
    global _NC
    if _NC is None:
        _NC = build_nc()
    return _NC


def _assemble(results):
    out = np.empty((4, 8, 128, 128), np.float32)
    for core in range(NCORES):
        b, half = divmod(core, 2)
        out[b, :, half * 64:half * 64 + 64, :] = results[core]["out"]
    return out


def kernel(_trace=False, **inputs):
    nc = _get_nc()
    in_maps = _make_in_maps(inputs)
    res = bass_utils.run_bass_kernel_spmd(
        nc, in_maps, core_ids=list(range(NCORES)), trace=_trace)
    out = _assemble(res.results)
    if _trace:
        return out, res
    return out


def kernel_sim(**inputs):
    """CoreSim-based check of a single core (core 0) against its slice."""
    from concourse.bass_interp import CoreSim
    nc = _get_nc()
    in_maps = _make_in_maps(inputs)
    outs = []
    for core in range(NCORES):
        sim = CoreSim(nc, trace=False, require_finite=False, require_nnan=False)
        for k, v in in_maps[core].items():
            sim.tensor(k)[:] = v
        sim.simulate()
        outs.append({"out": sim.tensor("out").copy()})
    return _assemble(outs)


# revision 7
# speedup vs baseline: 1.3043x; 1.2914x over previous
"""Trainium2 Bass kernel for nn_CollatedVanillaCNN.

The model applies a tiny CNN (log1p -> conv3x3(16->32)+bn+relu+avgpool2 ->
conv3x3(32->64)+bn+relu+avgpool2 -> fc(64->16)+bn+relu -> fc(16->8) -> expm1)
independently to the 4x4 sliding window at every pixel of x[4,16,128,128]
(zero-padded right/bottom), producing out[4,8,128,128].

Strategy: every output pixel is an independent sample => express the whole
network as 4 dense matmul stages over pixels (features on SBUF partitions,
pixels on the free dim):

  conv1 : windows  K=(sh,sw,c)=256  ->  M=(pw,qw,o1)=512   (masked 3x3 taps)
  conv2 : K=(pw,qw,o1)=512 -> M=(r,t,o2)=256               (avgpool1 folded in)
  fc1   : K=(r,t,o2)=256   -> M=16                         (avgpool2 folded in)
  fc2   : K=16 -> M=8

bn scales are folded into the weight columns; bn/conv biases are applied via
per-partition bias operands of the scalar-engine activation (relu / exp) or
vector-engine tensor_scalar.  Matmuls run as float32r (full-rate fp32).

Sharding: pure data parallel over B x H/2: core = (b, row half), 8192 pixels
per core, tiled as 16 tiles of 512 pixels (4 image rows).  Host does only
data movement (pad/im2col/layout); all arithmetic runs on device.
"""

import numpy as np

import concourse.bacc as bacc
import concourse.bass as bass
import concourse.mybir as mybir
import concourse.tile as tile
from concourse import bass_utils

AF = mybir.ActivationFunctionType
ALU = mybir.AluOpType
F32 = mybir.dt.float32
F32R = mybir.dt.float32r

EPS = 1e-5
NCORES = 8
NT = 16          # pixel tiles per core (each 4 image rows x 128 cols = 512 px)


# ---------------------------------------------------------------- host packing

def _pack_weights(p):
    """Pack all network params into device-layout matmul weights / biases."""
    w1 = p["conv1_w"].astype(np.float64)   # [32,16,3,3]
    w2 = p["conv2_w"].astype(np.float64)   # [64,32,3,3]
    s1 = (p["bn1_g"] / np.sqrt(p["bn1_v"] + EPS)).astype(np.float64)
    s2 = (p["bn2_g"] / np.sqrt(p["bn2_v"] + EPS)).astype(np.float64)
    s3 = (p["bn3_g"] / np.sqrt(p["bn3_v"] + EPS)).astype(np.float64)

    # conv1: rows (h,p) -> (sh,c,sw), cols f=(pw*4+qw)*32+o
    W1 = np.zeros((2, 128, 512), np.float64)
    pp = np.arange(128)
    c_of_p = (pp % 64) // 4
    sw_of_p = pp % 4
    f = np.arange(512)
    pw_of_f = f // 128
    qw_of_f = (f % 128) // 32
    o_of_f = f % 32
    for h in range(2):
        sh = 2 * h + pp // 64                       # [128]
        du = sh[:, None] - pw_of_f[None, :] + 1      # [128,512]
        dv = sw_of_p[:, None] - qw_of_f[None, :] + 1
        valid = (du >= 0) & (du < 3) & (dv >= 0) & (dv < 3)
        duc = np.clip(du, 0, 2)
        dvc = np.clip(dv, 0, 2)
        vals = w1[o_of_f[None, :].repeat(128, 0),
                  c_of_p[:, None].repeat(512, 1),
                  duc, dvc]
        W1[h] = np.where(valid, vals, 0.0) * s1[o_of_f][None, :]
    W1 = W1.transpose(1, 0, 2).reshape(128, 1024)    # [k, h*512+f]
    bias1 = ((p["conv1_b"] - p["bn1_m"]) * s1 + p["bn1_b"])  # [32] by o
    b1 = np.tile(bias1, 4).reshape(128, 1)           # partition (qw*32+o)

    # conv2 (+avgpool1): chunk pw, rows (qw*32+o1), cols f2=(r*2+t)*64+o2
    W2 = np.zeros((4, 128, 256), np.float64)
    row = np.arange(128)
    q_of_row = row // 32
    o1_of_row = row % 32
    f2 = np.arange(256)
    r_of_f2 = f2 // 128
    t_of_f2 = (f2 % 128) // 64
    o2_of_f2 = f2 % 64
    for pw in range(4):
        du = (pw // 2) - r_of_f2 + 1                 # [256] in {0,1,2}
        dv = (q_of_row // 2)[:, None] - t_of_f2[None, :] + 1
        vals = w2[o2_of_f2[None, :].repeat(128, 0),
                  o1_of_row[:, None].repeat(256, 1),
                  du[None, :].repeat(128, 0), dv]
        W2[pw] = 0.25 * vals * s2[o2_of_f2][None, :]
    W2 = W2.transpose(1, 0, 2).reshape(128, 1024)    # [row, pw*256+f2]
    bias2 = ((p["conv2_b"] - p["bn2_m"]) * s2 + p["bn2_b"])  # [64] by o2
    b2 = np.tile(bias2, 2).reshape(128, 1)           # partition (t*64+o2)

    # fc1 (+avgpool2): chunk r, rows (t*64+o2), cols m
    # rows (t,o2): value 0.25*fc1_w[m, o2]*s3[m], independent of (r,t)
    base = 0.25 * p["fc1_w"].astype(np.float64).T * s3[None, :]  # [64,16]
    W3 = np.stack([np.tile(base, (2, 1))] * 2)       # [2,128,16]
    W3 = W3.transpose(1, 0, 2).reshape(128, 32)      # [row, r*16+m]
    b3 = (((p["fc1_b"] - p["bn3_m"]) * s3 + p["bn3_b"]).reshape(16, 1))

    # fc2: rows m, cols o
    W4 = p["fc2_w"].astype(np.float64).T             # [16,8]
    b4 = p["fc2_b"].reshape(8, 1)

    return {
        "w1": np.ascontiguousarray(W1, np.float32),
        "w2": np.ascontiguousarray(W2, np.float32),
        "w3": np.ascontiguousarray(W3, np.float32),
        "w4": np.ascontiguousarray(W4, np.float32),
        "b1": np.ascontiguousarray(b1, np.float32),
        "b2": np.ascontiguousarray(b2, np.float32),
        "b3": np.ascontiguousarray(b3, np.float32),
        "b4": np.ascontiguousarray(b4, np.float32),
    }


def _im2col_core(xs):
    """xs: [16, 67, 131] padded row-slab -> xcols [16 tiles, 128, 1024]."""
    xcols = np.empty((NT, 128, 1024), np.float32)
    for sh in range(4):
        h, lo = divmod(sh, 2)
        for sw in range(4):
            blk = xs[:, sh:sh + 64, sw:sw + 128]         # [16c, 64, 128]
            blk = blk.reshape(16, NT, 4 * 128)           # [c, t, px]
            parts = 64 * lo + np.arange(16) * 4 + sw     # dest partitions
            xcols[:, parts, 512 * h:512 * h + 512] = blk.transpose(1, 0, 2)
    return xcols


def _make_in_maps(inputs):
    x = np.asarray(inputs["x"], np.float32)              # [4,16,128,128]
    xp = np.pad(x, ((0, 0), (0, 0), (0, 3), (0, 3)))     # [4,16,131,131]
    packed = _pack_weights({k: np.asarray(v, np.float64) for k, v in inputs.items()
                            if k != "x"})
    in_maps = []
    for core in range(NCORES):
        b, half = divmod(core, 2)
        r0 = half * 64
        xs = xp[b, :, r0:r0 + 67, :]
        m = dict(packed)
        m["xcols"] = _im2col_core(xs)
        in_maps.append(m)
    return in_maps


# ---------------------------------------------------------------- device build

def _pin_act_table_set():
    """Force every activation onto natural_log_exp_and_others (has Ln, Exp,
    Relu, Copy): the default per-function greedy set choice alternates table
    sets across Ln/Relu/Exp and burns ~2.7us per ACT_TABLE_LOAD, 28x."""
    from concourse.hw_specs import get_activation_tables as orig
    keep = "natural_log_exp_and_others"

    def patched(arch):
        t = orig(arch)
        return {name: (funcs if name == keep else set())
                for name, funcs in t.items()}

    bacc.get_activation_tables = patched


def build_nc():
    _pin_act_table_set()
    nc = bacc.Bacc("TRN2", target_bir_lowering=False, debug=False,
                   num_devices=NCORES)
    xcols_d = nc.dram_tensor("xcols", [NT, 128, 1024], F32, kind="ExternalInput")
    w1_d = nc.dram_tensor("w1", [128, 1024], F32R, kind="ExternalInput")
    w2_d = nc.dram_tensor("w2", [128, 1024], F32R, kind="ExternalInput")
    w3_d = nc.dram_tensor("w3", [128, 32], F32R, kind="ExternalInput")
    w4_d = nc.dram_tensor("w4", [16, 8], F32R, kind="ExternalInput")
    b1_d = nc.dram_tensor("b1", [128, 1], F32, kind="ExternalInput")
    b2_d = nc.dram_tensor("b2", [128, 1], F32, kind="ExternalInput")
    b3_d = nc.dram_tensor("b3", [16, 1], F32, kind="ExternalInput")
    b4_d = nc.dram_tensor("b4", [8, 1], F32, kind="ExternalInput")
    out_d = nc.dram_tensor("out", [8, 64, 128], F32, kind="ExternalOutput")

    with tile.TileContext(nc) as tc:
        with (
            tc.tile_pool(name="wpool", bufs=1) as wpool,
            tc.tile_pool(name="xin", bufs=3) as xin,
            tc.tile_pool(name="rhs1", bufs=2) as rhs1p,
            tc.tile_pool(name="relu1", bufs=2) as relu1p,
            tc.tile_pool(name="relu2", bufs=2) as relu2p,
            tc.tile_pool(name="relu3", bufs=2) as relu3p,
            tc.tile_pool(name="outsb", bufs=2) as outp,
            tc.tile_pool(name="ps1", bufs=2, space="PSUM") as ps1,
            tc.tile_pool(name="ps2", bufs=2, space="PSUM") as ps2,
            tc.tile_pool(name="psf1", bufs=1, space="PSUM") as psf1,
            tc.tile_pool(name="psf2", bufs=1, space="PSUM") as psf2,
        ):
            w1s = wpool.tile([128, 1024], F32R)
            w2s = wpool.tile([128, 1024], F32R)
            w3s = wpool.tile([128, 32], F32R)
            w4s = wpool.tile([16, 8], F32R)
            b1s = wpool.tile([128, 1], F32)
            b2s = wpool.tile([128, 1], F32)
            b3s = wpool.tile([16, 1], F32)
            b4s = wpool.tile([8, 1], F32)
            nc.sync.dma_start(w1s[:], w1_d[:])
            nc.sync.dma_start(w2s[:], w2_d[:])
            nc.sync.dma_start(w3s[:], w3_d[:])
            nc.sync.dma_start(w4s[:], w4_d[:])
            nc.sync.dma_start(b1s[:], b1_d[:])
            nc.sync.dma_start(b2s[:], b2_d[:])
            nc.sync.dma_start(b3s[:], b3_d[:])
            nc.sync.dma_start(b4s[:], b4_d[:])

            for t in range(NT):
                xt = xin.tile([128, 1024], F32)
                nc.sync.dma_start(xt[:], xcols_d[t])

                # log1p
                r1 = rhs1p.tile([128, 1024], F32R)
                nc.scalar.activation(r1[:], xt[:], AF.Ln, bias=1.0, scale=1.0)

                # conv1 (+bn1 scale): K=256, M=512.  Banded in (sh vs pw):
                # output chunk pw only needs window rows sh in
                # {pw-1,pw,pw+1} & [0,3], so 6 matmuls instead of 8.
                # rhs1 chunk h holds sh=2h (parts 0:64) / sh=2h+1 (64:128).
                rl1 = relu1p.tile([128, 2048], F32R)
                for half in range(2):           # halves: m in {0,1} / {2,3}
                    o1 = ps1.tile([128, 1024], F32)
                    if half == 0:
                        # m=0: sh{0,1} = chunk0 only
                        nc.tensor.matmul(o1[:, 0:512], w1s[:, 0:128],
                                         r1[:, 0:512], start=True, stop=True)
                        # m=1: sh{0,1,2} = chunk0 + lower half of chunk1
                        nc.tensor.matmul(o1[:, 512:1024], w1s[:, 128:256],
                                         r1[:, 0:512], start=True, stop=False)
                        nc.tensor.matmul(o1[:, 512:1024],
                                         w1s[0:64, 512 + 128:512 + 256],
                                         r1[0:64, 512:1024],
                                         start=False, stop=True)
                    else:
                        # m=2: sh{1,2,3} = upper half of chunk0 + chunk1
                        nc.tensor.matmul(o1[:, 0:512],
                                         w1s[64:128, 256:384],
                                         r1[64:128, 0:512],
                                         start=True, stop=False)
                        nc.tensor.matmul(o1[:, 0:512],
                                         w1s[:, 512 + 256:512 + 384],
                                         r1[:, 512:1024], start=False,
                                         stop=True)
                        # m=3: sh{2,3} = chunk1 only
                        nc.tensor.matmul(o1[:, 512:1024],
                                         w1s[:, 512 + 384:512 + 512],
                                         r1[:, 512:1024], start=True,
                                         stop=True)
                    # bn1 bias + relu on this half
                    nc.scalar.activation(rl1[:, 1024 * half:1024 * (half + 1)],
                                         o1[:], AF.Relu, bias=b1s[:],
                                         scale=1.0)

                # conv2 (+pool1, +bn2 scale): K=512 (4 chunks), M=256 (2 chunks)
                rl2 = relu2p.tile([128, 1024], F32R)
                for n in range(2):
                    o2 = ps2.tile([128, 512], F32)
                    for pw in range(4):
                        nc.tensor.matmul(
                            o2[:],
                            w2s[:, 256 * pw + 128 * n:256 * pw + 128 * n + 128],
                            rl1[:, 512 * pw:512 * pw + 512],
                            start=(pw == 0), stop=(pw == 3),
                        )
                    # bn2 bias + relu (vector engine)
                    nc.vector.tensor_scalar(rl2[:, 512 * n:512 * (n + 1)],
                                            o2[:], b2s[:], 0.0,
                                            op0=ALU.add, op1=ALU.max)

                # fc1 (+pool2, +bn3 scale): K=256 (2 chunks), M=16
                f1 = psf1.tile([16, 512], F32)
                for h in range(2):
                    nc.tensor.matmul(
                        f1[:],
                        w3s[:, 16 * h:16 * h + 16],
                        rl2[:, 512 * h:512 * h + 512],
                        start=(h == 0), stop=(h == 1),
                    )
                rl3 = relu3p.tile([16, 512], F32R)
                nc.vector.tensor_scalar(rl3[:], f1[:], b3s[:], 0.0,
                                        op0=ALU.add, op1=ALU.max)

                # fc2: K=16, M=8
                f2 = psf2.tile([8, 512], F32)
                nc.tensor.matmul(f2[:], w4s[:], rl3[:],
                                 start=True, stop=True)

                # expm1 = exp(x + fc2_b) - 1
                ob = outp.tile([8, 512], F32, tag="ob")
                nc.scalar.activation(ob[:], f2[:], AF.Exp, bias=b4s[:],
                                     scale=1.0)
                ob2 = outp.tile([8, 512], F32, tag="ob2")
                nc.vector.tensor_scalar(ob2[:], ob[:], 1.0, None,
                                        op0=ALU.subtract)

                nc.sync.dma_start(out_d[:, 4 * t:4 * t + 4, :], ob2[:])

    nc.compile()
    return nc


_NC = None


def _get_nc():
    global _NC
    if _NC is None:
        _NC = build_nc()
    return _NC


def _assemble(results):
    out = np.empty((4, 8, 128, 128), np.float32)
    for core in range(NCORES):
        b, half = divmod(core, 2)
        out[b, :, half * 64:half * 64 + 64, :] = results[core]["out"]
    return out


def kernel(_trace=False, **inputs):
    nc = _get_nc()
    in_maps = _make_in_maps(inputs)
    res = bass_utils.run_bass_kernel_spmd(
        nc, in_maps, core_ids=list(range(NCORES)), trace=_trace)
    out = _assemble(res.results)
    if _trace:
        return out, res
    return out


def kernel_sim(**inputs):
    """CoreSim-based check of a single core (core 0) against its slice."""
    from concourse.bass_interp import CoreSim
    nc = _get_nc()
    in_maps = _make_in_maps(inputs)
    outs = []
    for core in range(NCORES):
        sim = CoreSim(nc, trace=False, require_finite=False, require_nnan=False)
        for k, v in in_maps[core].items():
            sim.tensor(k)[:] = v
        sim.simulate()
        outs.append({"out": sim.tensor("out").copy()})
    return _assemble(outs)


# revision 11
# speedup vs baseline: 1.4107x; 1.0816x over previous
"""Trainium2 Bass kernel for nn_CollatedVanillaCNN.

The model applies a tiny CNN (log1p -> conv3x3(16->32)+bn+relu+avgpool2 ->
conv3x3(32->64)+bn+relu+avgpool2 -> fc(64->16)+bn+relu -> fc(16->8) -> expm1)
independently to the 4x4 sliding window at every pixel of x[4,16,128,128]
(zero-padded right/bottom), producing out[4,8,128,128].

Strategy: every output pixel is an independent sample => express the whole
network as 4 dense matmul stages over pixels (features on SBUF partitions,
pixels on the free dim):

  conv1 : windows  K=(sh,sw,c)=256  ->  M=(pw,qw,o1)=512   (masked 3x3 taps)
  conv2 : K=(pw,qw,o1)=512 -> M=(r,t,o2)=256               (avgpool1 folded in)
  fc1   : K=(r,t,o2)=256   -> M=16                         (avgpool2 folded in)
  fc2   : K=16 -> M=8

bn scales are folded into the weight columns; bn/conv biases are applied via
per-partition bias operands of the scalar-engine activation (relu / exp) or
vector-engine tensor_scalar.  Matmuls run as float32r (full-rate fp32).

Sharding: pure data parallel over B x H/2: core = (b, row half), 8192 pixels
per core, tiled as 16 tiles of 512 pixels (4 image rows).  Host does only
data movement (pad/im2col/layout); all arithmetic runs on device.
"""

import numpy as np

import concourse.bacc as bacc
import concourse.bass as bass
import concourse.mybir as mybir
import concourse.tile as tile
from concourse import bass_utils

AF = mybir.ActivationFunctionType
ALU = mybir.AluOpType
F32 = mybir.dt.float32
F32R = mybir.dt.float32r

EPS = 1e-5
NCORES = 8
NT = 16          # pixel tiles per core (each 4 image rows x 128 cols = 512 px)


# ---------------------------------------------------------------- host packing

def _pack_weights(p):
    """Pack all network params into device-layout matmul weights / biases."""
    w1 = p["conv1_w"].astype(np.float64)   # [32,16,3,3]
    w2 = p["conv2_w"].astype(np.float64)   # [64,32,3,3]
    s1 = (p["bn1_g"] / np.sqrt(p["bn1_v"] + EPS)).astype(np.float64)
    s2 = (p["bn2_g"] / np.sqrt(p["bn2_v"] + EPS)).astype(np.float64)
    s3 = (p["bn3_g"] / np.sqrt(p["bn3_v"] + EPS)).astype(np.float64)

    # conv1: rows (h,p) -> (sh,c,sw), cols f=(pw*4+qw)*32+o
    W1 = np.zeros((2, 128, 512), np.float64)
    pp = np.arange(128)
    c_of_p = (pp % 64) // 4
    sw_of_p = pp % 4
    f = np.arange(512)
    pw_of_f = f // 128
    qw_of_f = (f % 128) // 32
    o_of_f = f % 32
    for h in range(2):
        sh = 2 * h + pp // 64                       # [128]
        du = sh[:, None] - pw_of_f[None, :] + 1      # [128,512]
        dv = sw_of_p[:, None] - qw_of_f[None, :] + 1
        valid = (du >= 0) & (du < 3) & (dv >= 0) & (dv < 3)
        duc = np.clip(du, 0, 2)
        dvc = np.clip(dv, 0, 2)
        vals = w1[o_of_f[None, :].repeat(128, 0),
                  c_of_p[:, None].repeat(512, 1),
                  duc, dvc]
        W1[h] = np.where(valid, vals, 0.0) * s1[o_of_f][None, :]
    W1 = W1.transpose(1, 0, 2).reshape(128, 1024)    # [k, h*512+f]
    bias1 = ((p["conv1_b"] - p["bn1_m"]) * s1 + p["bn1_b"])  # [32] by o
    b1 = np.tile(bias1, 4).reshape(128, 1)           # partition (qw*32+o)

    # conv2 (+avgpool1): chunk pw, rows (qw*32+o1), cols f2=(r*2+t)*64+o2
    W2 = np.zeros((4, 128, 256), np.float64)
    row = np.arange(128)
    q_of_row = row // 32
    o1_of_row = row % 32
    f2 = np.arange(256)
    r_of_f2 = f2 // 128
    t_of_f2 = (f2 % 128) // 64
    o2_of_f2 = f2 % 64
    for pw in range(4):
        du = (pw // 2) - r_of_f2 + 1                 # [256] in {0,1,2}
        dv = (q_of_row // 2)[:, None] - t_of_f2[None, :] + 1
        vals = w2[o2_of_f2[None, :].repeat(128, 0),
                  o1_of_row[:, None].repeat(256, 1),
                  du[None, :].repeat(128, 0), dv]
        W2[pw] = 0.25 * vals * s2[o2_of_f2][None, :]
    W2 = W2.transpose(1, 0, 2).reshape(128, 1024)    # [row, pw*256+f2]
    bias2 = ((p["conv2_b"] - p["bn2_m"]) * s2 + p["bn2_b"])  # [64] by o2
    b2 = np.tile(bias2, 2).reshape(128, 1)           # partition (t*64+o2)

    # fc1 (+avgpool2): chunk r, rows (t*64+o2), cols m
    # rows (t,o2): value 0.25*fc1_w[m, o2]*s3[m], independent of (r,t)
    base = 0.25 * p["fc1_w"].astype(np.float64).T * s3[None, :]  # [64,16]
    W3 = np.stack([np.tile(base, (2, 1))] * 2)       # [2,128,16]
    W3 = W3.transpose(1, 0, 2).reshape(128, 32)      # [row, r*16+m]
    b3 = (((p["fc1_b"] - p["bn3_m"]) * s3 + p["bn3_b"]).reshape(16, 1))

    # fc2: rows m, cols o
    W4 = p["fc2_w"].astype(np.float64).T             # [16,8]
    b4 = p["fc2_b"].reshape(8, 1)

    return {
        "w1": np.ascontiguousarray(W1, np.float32),
        "w2": np.ascontiguousarray(W2, np.float32),
        "w3": np.ascontiguousarray(W3, np.float32),
        "w4": np.ascontiguousarray(W4, np.float32),
        "b1": np.ascontiguousarray(b1, np.float32),
        "b2": np.ascontiguousarray(b2, np.float32),
        "b3": np.ascontiguousarray(b3, np.float32),
        "b4": np.ascontiguousarray(b4, np.float32),
    }


def _im2col_core(xs):
    """xs: [16, 67, 131] padded row-slab -> xcols [16 tiles, 128, 1024]."""
    xcols = np.empty((NT, 128, 1024), np.float32)
    for sh in range(4):
        h, lo = divmod(sh, 2)
        for sw in range(4):
            blk = xs[:, sh:sh + 64, sw:sw + 128]         # [16c, 64, 128]
            blk = blk.reshape(16, NT, 4 * 128)           # [c, t, px]
            parts = 64 * lo + np.arange(16) * 4 + sw     # dest partitions
            xcols[:, parts, 512 * h:512 * h + 512] = blk.transpose(1, 0, 2)
    return xcols


def _make_in_maps(inputs):
    x = np.asarray(inputs["x"], np.float32)              # [4,16,128,128]
    xp = np.pad(x, ((0, 0), (0, 0), (0, 3), (0, 3)))     # [4,16,131,131]
    packed = _pack_weights({k: np.asarray(v, np.float64) for k, v in inputs.items()
                            if k != "x"})
    in_maps = []
    for core in range(NCORES):
        b, half = divmod(core, 2)
        r0 = half * 64
        xs = xp[b, :, r0:r0 + 67, :]
        m = dict(packed)
        m["xcols"] = _im2col_core(xs)
        in_maps.append(m)
    return in_maps


# ---------------------------------------------------------------- device build

def _pin_act_table_set():
    """Force every activation onto natural_log_exp_and_others (has Ln, Exp,
    Relu, Copy): the default per-function greedy set choice alternates table
    sets across Ln/Relu/Exp and burns ~2.7us per ACT_TABLE_LOAD, 28x."""
    from concourse.hw_specs import get_activation_tables as orig
    keep = "natural_log_exp_and_others"

    def patched(arch):
        t = orig(arch)
        return {name: (funcs if name == keep else set())
                for name, funcs in t.items()}

    bacc.get_activation_tables = patched


def build_nc():
    _pin_act_table_set()
    nc = bacc.Bacc("TRN2", target_bir_lowering=False, debug=False,
                   num_devices=NCORES)
    xcols_d = nc.dram_tensor("xcols", [NT, 128, 1024], F32, kind="ExternalInput")
    w1_d = nc.dram_tensor("w1", [128, 1024], F32R, kind="ExternalInput")
    w2_d = nc.dram_tensor("w2", [128, 1024], F32R, kind="ExternalInput")
    w3_d = nc.dram_tensor("w3", [128, 32], F32R, kind="ExternalInput")
    w4_d = nc.dram_tensor("w4", [16, 8], F32R, kind="ExternalInput")
    b1_d = nc.dram_tensor("b1", [128, 1], F32, kind="ExternalInput")
    b2_d = nc.dram_tensor("b2", [128, 1], F32, kind="ExternalInput")
    b3_d = nc.dram_tensor("b3", [16, 1], F32, kind="ExternalInput")
    b4_d = nc.dram_tensor("b4", [8, 1], F32, kind="ExternalInput")
    out_d = nc.dram_tensor("out", [8, 64, 128], F32, kind="ExternalOutput")

    with tile.TileContext(nc) as tc:
        with (
            tc.tile_pool(name="wpool", bufs=1) as wpool,
            tc.tile_pool(name="xin", bufs=3) as xin,
            tc.tile_pool(name="rhs1", bufs=2) as rhs1p,
            tc.tile_pool(name="relu1", bufs=2) as relu1p,
            tc.tile_pool(name="relu2", bufs=2) as relu2p,
            tc.tile_pool(name="relu3", bufs=2) as relu3p,
            tc.tile_pool(name="outsb", bufs=2) as outp,
            tc.tile_pool(name="ps1", bufs=2, space="PSUM") as ps1,
            tc.tile_pool(name="ps2", bufs=2, space="PSUM") as ps2,
            tc.tile_pool(name="psf1", bufs=1, space="PSUM") as psf1,
            tc.tile_pool(name="psf2", bufs=1, space="PSUM") as psf2,
        ):
            w1s = wpool.tile([128, 1024], F32R)
            w2s = wpool.tile([128, 1024], F32R)
            w3s = wpool.tile([128, 32], F32R)
            w4s = wpool.tile([16, 8], F32R)
            b1s = wpool.tile([128, 1], F32)
            b2s = wpool.tile([128, 1], F32)
            b3s = wpool.tile([16, 1], F32)
            b4s = wpool.tile([8, 1], F32)
            nc.sync.dma_start(w1s[:], w1_d[:])
            nc.sync.dma_start(w2s[:], w2_d[:])
            nc.sync.dma_start(w3s[:], w3_d[:])
            nc.sync.dma_start(w4s[:], w4_d[:])
            nc.sync.dma_start(b1s[:], b1_d[:])
            nc.sync.dma_start(b2s[:], b2_d[:])
            nc.sync.dma_start(b3s[:], b3_d[:])
            nc.sync.dma_start(b4s[:], b4_d[:])

            # Software-pipelined over tiles with a 3-stage skew so every
            # engine always has cross-tile work queued in its program order:
            #   front(t0): load, log1p, conv1, bn1
            #   mid(t1=t0-1): conv2, bn2
            #   back(t2=t0-2): fc1, bn3, fc2, exp, -1, store
            xts = {}
            r1s = {}
            rl1s = {}
            rl2s = {}

            def dma_in(t):
                xts[t] = xin.tile([128, 1024], F32, name="xt", tag="xt")
                nc.sync.dma_start(xts[t][:], xcols_d[t])

            def front(t):
                r1s[t] = rhs1p.tile([128, 1024], F32R, name="r1", tag="r1")
                nc.scalar.activation(r1s[t][:], xts[t][:], AF.Ln,
                                     bias=1.0, scale=1.0)
                del xts[t]

                # conv1 (+bn1 scale): K=256, M=512.  Banded in (sh vs pw):
                # output chunk pw only needs window rows sh in
                # {pw-1,pw,pw+1} & [0,3], so 6 matmuls instead of 8.
                # rhs1 chunk h holds sh=2h (parts 0:64) / sh=2h+1 (64:128).
                r1 = r1s[t]
                rl1s[t] = rl1 = relu1p.tile([128, 2048], F32R, name="rl1", tag="rl1")
                for half in range(2):           # halves: m in {0,1} / {2,3}
                    o1 = ps1.tile([128, 1024], F32, tag="o1")
                    if half == 0:
                        # m=0: sh{0,1} = chunk0 only
                        nc.tensor.matmul(o1[:, 0:512], w1s[:, 0:128],
                                         r1[:, 0:512], start=True, stop=True)
                        # m=1: sh{0,1,2} = chunk0 + lower half of chunk1
                        nc.tensor.matmul(o1[:, 512:1024], w1s[:, 128:256],
                                         r1[:, 0:512], start=True, stop=False)
                        nc.tensor.matmul(o1[:, 512:1024],
                                         w1s[0:64, 512 + 128:512 + 256],
                                         r1[0:64, 512:1024],
                                         start=False, stop=True)
                    else:
                        # m=2: sh{1,2,3} = upper half of chunk0 + chunk1
                        nc.tensor.matmul(o1[:, 0:512],
                                         w1s[64:128, 256:384],
                                         r1[64:128, 0:512],
                                         start=True, stop=False)
                        nc.tensor.matmul(o1[:, 0:512],
                                         w1s[:, 512 + 256:512 + 384],
                                         r1[:, 512:1024], start=False,
                                         stop=True)
                        # m=3: sh{2,3} = chunk1 only
                        nc.tensor.matmul(o1[:, 512:1024],
                                         w1s[:, 512 + 384:512 + 512],
                                         r1[:, 512:1024], start=True,
                                         stop=True)
                    # bn1 bias + relu on this half
                    nc.scalar.activation(rl1[:, 1024 * half:1024 * (half + 1)],
                                         o1[:], AF.Relu, bias=b1s[:],
                                         scale=1.0)
                del r1s[t]

            def mid(t):
                # conv2 (+pool1, +bn2 scale): K=512 (4 chunks), M=256 (2)
                rl1 = rl1s[t]
                rl2s[t] = rl2 = relu2p.tile([128, 1024], F32R, name="rl2", tag="rl2")
                for n in range(2):
                    o2 = ps2.tile([128, 512], F32, tag="o2")
                    for pw in range(4):
                        nc.tensor.matmul(
                            o2[:],
                            w2s[:, 256 * pw + 128 * n:256 * pw + 128 * n + 128],
                            rl1[:, 512 * pw:512 * pw + 512],
                            start=(pw == 0), stop=(pw == 3),
                        )
                    # bn2 bias + relu (vector engine)
                    nc.vector.tensor_scalar(rl2[:, 512 * n:512 * (n + 1)],
                                            o2[:], b2s[:], 0.0,
                                            op0=ALU.add, op1=ALU.max)
                del rl1s[t]

            def back_fc1(t):
                # fc1 (+pool2, +bn3 scale): K=256 (2 chunks), M=16
                rl2 = rl2s[t]
                f1 = psf1.tile([16, 512], F32, tag="f1")
                for h in range(2):
                    nc.tensor.matmul(
                        f1[:],
                        w3s[:, 16 * h:16 * h + 16],
                        rl2[:, 512 * h:512 * h + 512],
                        start=(h == 0), stop=(h == 1),
                    )
                rl3 = relu3p.tile([16, 512], F32R, tag="rl3")
                nc.vector.tensor_scalar(rl3[:], f1[:], b3s[:], 0.0,
                                        op0=ALU.add, op1=ALU.max)
                del rl2s[t]
                return rl3

            def back_fc2(t, rl3):
                # fc2: K=16, M=8
                f2 = psf2.tile([8, 512], F32, tag="f2")
                nc.tensor.matmul(f2[:], w4s[:], rl3[:],
                                 start=True, stop=True)
                # expm1 = exp(x + fc2_b) - 1
                ob = outp.tile([8, 512], F32, tag="ob")
                nc.scalar.activation(ob[:], f2[:], AF.Exp, bias=b4s[:],
                                     scale=1.0)
                ob2 = outp.tile([8, 512], F32, tag="ob2")
                nc.vector.tensor_scalar(ob2[:], ob[:], 1.0, None,
                                        op0=ALU.subtract)
                nc.sync.dma_start(out_d[:, 4 * t:4 * t + 4, :], ob2[:])

            dma_in(0)
            for s in range(NT + 2):
                t0, t1, t2 = s, s - 1, s - 2
                if s + 1 < NT:
                    dma_in(s + 1)
                rl3 = None
                if 0 <= t1 < NT:
                    mid(t1)
                if 0 <= t2 < NT:
                    rl3 = back_fc1(t2)
                if t0 < NT:
                    front(t0)
                if 0 <= t2 < NT:
                    back_fc2(t2, rl3)

    nc.compile()
    return nc


_NC = None


def _get_nc():
    global _NC
    if _NC is None:
        _NC = build_nc()
    return _NC


def _assemble(results):
    out = np.empty((4, 8, 128, 128), np.float32)
    for core in range(NCORES):
        b, half = divmod(core, 2)
        out[b, :, half * 64:half * 64 + 64, :] = results[core]["out"]
    return out


def kernel(_trace=False, **inputs):
    nc = _get_nc()
    in_maps = _make_in_maps(inputs)
    res = bass_utils.run_bass_kernel_spmd(
        nc, in_maps, core_ids=list(range(NCORES)), trace=_trace)
    out = _assemble(res.results)
    if _trace:
        return out, res
    return out


def kernel_sim(**inputs):
    """CoreSim-based check of a single core (core 0) against its slice."""
    from concourse.bass_interp import CoreSim
    nc = _get_nc()
    in_maps = _make_in_maps(inputs)
    outs = []
    for core in range(NCORES):
        sim = CoreSim(nc, trace=False, require_finite=False, require_nnan=False)
        for k, v in in_maps[core].items():
            sim.tensor(k)[:] = v
        sim.simulate()
        outs.append({"out": sim.tensor("out").copy()})
    return _assemble(outs)


# revision 13
# speedup vs baseline: 1.5088x; 1.0695x over previous
"""Trainium2 Bass kernel for nn_CollatedVanillaCNN.

The model applies a tiny CNN (log1p -> conv3x3(16->32)+bn+relu+avgpool2 ->
conv3x3(32->64)+bn+relu+avgpool2 -> fc(64->16)+bn+relu -> fc(16->8) -> expm1)
independently to the 4x4 sliding window at every pixel of x[4,16,128,128]
(zero-padded right/bottom), producing out[4,8,128,128].

Strategy: every output pixel is an independent sample => express the whole
network as 4 dense matmul stages over pixels (features on SBUF partitions,
pixels on the free dim):

  conv1 : windows  K=(sh,sw,c)=256  ->  M=(pw,qw,o1)=512   (masked 3x3 taps)
  conv2 : K=(pw,qw,o1)=512 -> M=(r,t,o2)=256               (avgpool1 folded in)
  fc1   : K=(r,t,o2)=256   -> M=16                         (avgpool2 folded in)
  fc2   : K=16 -> M=8

bn scales are folded into the weight columns; bn/conv biases are applied via
per-partition bias operands of the scalar-engine activation (relu / exp) or
vector-engine tensor_scalar.  Matmuls run as float32r (full-rate fp32).

Sharding: pure data parallel over B x H/2: core = (b, row half), 8192 pixels
per core, tiled as 16 tiles of 512 pixels (4 image rows).  Host does only
data movement (pad/im2col/layout); all arithmetic runs on device.
"""

import numpy as np

import concourse.bacc as bacc
import concourse.bass as bass
import concourse.mybir as mybir
import concourse.tile as tile
from concourse import bass_utils

AF = mybir.ActivationFunctionType
ALU = mybir.AluOpType
F32 = mybir.dt.float32
F32R = mybir.dt.float32r

EPS = 1e-5
NCORES = 8
NT = 16          # pixel tiles per core (each 4 image rows x 128 cols = 512 px)


# ---------------------------------------------------------------- host packing

def _pack_weights(p):
    """Pack all network params into device-layout matmul weights / biases."""
    w1 = p["conv1_w"].astype(np.float64)   # [32,16,3,3]
    w2 = p["conv2_w"].astype(np.float64)   # [64,32,3,3]
    s1 = (p["bn1_g"] / np.sqrt(p["bn1_v"] + EPS)).astype(np.float64)
    s2 = (p["bn2_g"] / np.sqrt(p["bn2_v"] + EPS)).astype(np.float64)
    s3 = (p["bn3_g"] / np.sqrt(p["bn3_v"] + EPS)).astype(np.float64)

    # conv1: rows (h,p) -> (sh,c,sw), cols f=(pw*4+qw)*32+o
    W1 = np.zeros((2, 128, 512), np.float64)
    pp = np.arange(128)
    c_of_p = (pp % 64) // 4
    sw_of_p = pp % 4
    f = np.arange(512)
    pw_of_f = f // 128
    qw_of_f = (f % 128) // 32
    o_of_f = f % 32
    for h in range(2):
        sh = 2 * h + pp // 64                       # [128]
        du = sh[:, None] - pw_of_f[None, :] + 1      # [128,512]
        dv = sw_of_p[:, None] - qw_of_f[None, :] + 1
        valid = (du >= 0) & (du < 3) & (dv >= 0) & (dv < 3)
        duc = np.clip(du, 0, 2)
        dvc = np.clip(dv, 0, 2)
        vals = w1[o_of_f[None, :].repeat(128, 0),
                  c_of_p[:, None].repeat(512, 1),
                  duc, dvc]
        W1[h] = np.where(valid, vals, 0.0) * s1[o_of_f][None, :]
    W1 = W1.transpose(1, 0, 2).reshape(128, 1024)    # [k, h*512+f]
    bias1 = ((p["conv1_b"] - p["bn1_m"]) * s1 + p["bn1_b"])  # [32] by o
    b1 = np.tile(bias1, 4).reshape(128, 1)           # partition (qw*32+o)

    # conv2 (+avgpool1): chunk pw, rows (qw*32+o1), cols f2=(r*2+t)*64+o2
    W2 = np.zeros((4, 128, 256), np.float64)
    row = np.arange(128)
    q_of_row = row // 32
    o1_of_row = row % 32
    f2 = np.arange(256)
    r_of_f2 = f2 // 128
    t_of_f2 = (f2 % 128) // 64
    o2_of_f2 = f2 % 64
    for pw in range(4):
        du = (pw // 2) - r_of_f2 + 1                 # [256] in {0,1,2}
        dv = (q_of_row // 2)[:, None] - t_of_f2[None, :] + 1
        vals = w2[o2_of_f2[None, :].repeat(128, 0),
                  o1_of_row[:, None].repeat(256, 1),
                  du[None, :].repeat(128, 0), dv]
        W2[pw] = 0.25 * vals * s2[o2_of_f2][None, :]
    W2 = W2.transpose(1, 0, 2).reshape(128, 1024)    # [row, pw*256+f2]
    bias2 = ((p["conv2_b"] - p["bn2_m"]) * s2 + p["bn2_b"])  # [64] by o2
    b2 = np.tile(bias2, 2).reshape(128, 1)           # partition (t*64+o2)

    # fc1 (+avgpool2): chunk r, rows (t*64+o2), cols m
    # rows (t,o2): value 0.25*fc1_w[m, o2]*s3[m], independent of (r,t)
    base = 0.25 * p["fc1_w"].astype(np.float64).T * s3[None, :]  # [64,16]
    W3 = np.stack([np.tile(base, (2, 1))] * 2)       # [2,128,16]
    W3 = W3.transpose(1, 0, 2).reshape(128, 32)      # [row, r*16+m]
    b3 = (((p["fc1_b"] - p["bn3_m"]) * s3 + p["bn3_b"]).reshape(16, 1))

    # fc2: rows m, cols o
    W4 = p["fc2_w"].astype(np.float64).T             # [16,8]
    b4 = p["fc2_b"].reshape(8, 1)

    # Combined device layouts: one weight tensor (f32r) + one bias tensor.
    wtot = np.zeros((128, 1024 + 1024 + 32 + 8), np.float64)
    wtot[:, 0:1024] = W1
    wtot[:, 1024:2048] = W2
    wtot[:, 2048:2080] = W3
    wtot[0:16, 2080:2088] = W4
    btot = np.zeros((128, 4), np.float64)
    btot[:, 0:1] = b1
    btot[:, 1:2] = b2
    btot[0:16, 2:3] = b3
    btot[0:8, 3:4] = b4
    return {
        "wtot": np.ascontiguousarray(wtot, np.float32),
        "btot": np.ascontiguousarray(btot, np.float32),
    }


def _im2col_core(xs):
    """xs: [16, 67, 131] padded row-slab -> xcols [16 tiles, 128, 1024]."""
    xcols = np.empty((NT, 128, 1024), np.float32)
    for sh in range(4):
        h, lo = divmod(sh, 2)
        for sw in range(4):
            blk = xs[:, sh:sh + 64, sw:sw + 128]         # [16c, 64, 128]
            blk = blk.reshape(16, NT, 4 * 128)           # [c, t, px]
            parts = 64 * lo + np.arange(16) * 4 + sw     # dest partitions
            xcols[:, parts, 512 * h:512 * h + 512] = blk.transpose(1, 0, 2)
    return xcols


def _make_in_maps(inputs):
    x = np.asarray(inputs["x"], np.float32)              # [4,16,128,128]
    xp = np.pad(x, ((0, 0), (0, 0), (0, 3), (0, 3)))     # [4,16,131,131]
    packed = _pack_weights({k: np.asarray(v, np.float64) for k, v in inputs.items()
                            if k != "x"})
    in_maps = []
    for core in range(NCORES):
        b, half = divmod(core, 2)
        r0 = half * 64
        xs = xp[b, :, r0:r0 + 67, :]
        m = dict(packed)
        m["xcols"] = _im2col_core(xs)
        in_maps.append(m)
    return in_maps


# ---------------------------------------------------------------- device build

def _pin_act_table_set():
    """Force every activation onto natural_log_exp_and_others (has Ln, Exp,
    Relu, Copy): the default per-function greedy set choice alternates table
    sets across Ln/Relu/Exp and burns ~2.7us per ACT_TABLE_LOAD, 28x."""
    from concourse.hw_specs import get_activation_tables as orig
    keep = "natural_log_exp_and_others"

    def patched(arch):
        t = orig(arch)
        return {name: (funcs if name == keep else set())
                for name, funcs in t.items()}

    bacc.get_activation_tables = patched


def build_nc():
    _pin_act_table_set()
    nc = bacc.Bacc("TRN2", target_bir_lowering=False, debug=False,
                   num_devices=NCORES)
    xcols_d = nc.dram_tensor("xcols", [NT, 128, 1024], F32, kind="ExternalInput")
    wtot_d = nc.dram_tensor("wtot", [128, 2088], F32R, kind="ExternalInput")
    btot_d = nc.dram_tensor("btot", [128, 4], F32, kind="ExternalInput")
    out_d = nc.dram_tensor("out", [8, 64, 128], F32, kind="ExternalOutput")

    with tile.TileContext(nc) as tc:
        with (
            tc.tile_pool(name="wpool", bufs=1) as wpool,
            tc.tile_pool(name="xin", bufs=3) as xin,
            tc.tile_pool(name="rhs1", bufs=2) as rhs1p,
            tc.tile_pool(name="relu1", bufs=2) as relu1p,
            tc.tile_pool(name="relu2", bufs=2) as relu2p,
            tc.tile_pool(name="relu3", bufs=2) as relu3p,
            tc.tile_pool(name="outsb", bufs=2) as outp,
            tc.tile_pool(name="ps1", bufs=2, space="PSUM") as ps1,
            tc.tile_pool(name="ps2", bufs=2, space="PSUM") as ps2,
            tc.tile_pool(name="psf1", bufs=1, space="PSUM") as psf1,
            tc.tile_pool(name="psf2", bufs=1, space="PSUM") as psf2,
        ):
            ws = wpool.tile([128, 2088], F32R)
            bs = wpool.tile([128, 4], F32)
            w1s = ws[:, 0:1024]
            w2s = ws[:, 1024:2048]
            w3s = ws[:, 2048:2080]
            w4s = ws[0:16, 2080:2088]
            b1s = bs[:, 0:1]
            b2s = bs[:, 1:2]
            b3s = bs[0:16, 2:3]
            b4s = bs[0:8, 3:4]

            # Software-pipelined over tiles with a 3-stage skew so every
            # engine always has cross-tile work queued in its program order:
            #   front(t0): load, log1p, conv1, bn1
            #   mid(t1=t0-1): conv2, bn2
            #   back(t2=t0-2): fc1, bn3, fc2, exp, -1, store
            xts = {}
            r1s = {}
            rl1s = {}
            rl2s = {}

            def dma_in(t):
                xts[t] = xin.tile([128, 1024], F32, name="xt", tag="xt")
                nc.sync.dma_start(xts[t][:], xcols_d[t])

            def front(t):
                r1s[t] = rhs1p.tile([128, 1024], F32R, name="r1", tag="r1")
                nc.scalar.activation(r1s[t][:], xts[t][:], AF.Ln,
                                     bias=1.0, scale=1.0)
                del xts[t]

                # conv1 (+bn1 scale): K=256, M=512.  Banded in (sh vs pw):
                # output chunk pw only needs window rows sh in
                # {pw-1,pw,pw+1} & [0,3], so 6 matmuls instead of 8.
                # rhs1 chunk h holds sh=2h (parts 0:64) / sh=2h+1 (64:128).
                r1 = r1s[t]
                rl1s[t] = rl1 = relu1p.tile([128, 2048], F32R, name="rl1", tag="rl1")
                for half in range(2):           # halves: m in {0,1} / {2,3}
                    o1 = ps1.tile([128, 1024], F32, tag="o1")
                    if half == 0:
                        # m=0: sh{0,1} = chunk0 only
                        nc.tensor.matmul(o1[:, 0:512], w1s[:, 0:128],
                                         r1[:, 0:512], start=True, stop=True)
                        # m=1: sh{0,1,2}; chunk1 rows sh=3 are zero weights
                        nc.tensor.matmul(o1[:, 512:1024], w1s[:, 128:256],
                                         r1[:, 0:512], start=True, stop=False)
                        nc.tensor.matmul(o1[:, 512:1024],
                                         w1s[:, 512 + 128:512 + 256],
                                         r1[:, 512:1024],
                                         start=False, stop=True)
                    else:
                        # m=2: sh{1,2,3}; chunk0 rows sh=0 are zero weights
                        nc.tensor.matmul(o1[:, 0:512],
                                         w1s[:, 256:384],
                                         r1[:, 0:512],
                                         start=True, stop=False)
                        nc.tensor.matmul(o1[:, 0:512],
                                         w1s[:, 512 + 256:512 + 384],
                                         r1[:, 512:1024], start=False,
                                         stop=True)
                        # m=3: sh{2,3} = chunk1 only
                        nc.tensor.matmul(o1[:, 512:1024],
                                         w1s[:, 512 + 384:512 + 512],
                                         r1[:, 512:1024], start=True,
                                         stop=True)
                    # bn1 bias + relu on this half
                    nc.scalar.activation(rl1[:, 1024 * half:1024 * (half + 1)],
                                         o1[:], AF.Relu, bias=b1s[:],
                                         scale=1.0)
                del r1s[t]

            def mid(t):
                # conv2 (+pool1, +bn2 scale): K=512 (4 chunks), M=256 (2)
                rl1 = rl1s[t]
                rl2s[t] = rl2 = relu2p.tile([128, 1024], F32R, name="rl2", tag="rl2")
                for n in range(2):
                    o2 = ps2.tile([128, 512], F32, tag="o2")
                    for pw in range(4):
                        nc.tensor.matmul(
                            o2[:],
                            w2s[:, 256 * pw + 128 * n:256 * pw + 128 * n + 128],
                            rl1[:, 512 * pw:512 * pw + 512],
                            start=(pw == 0), stop=(pw == 3),
                        )
                    # bn2 bias + relu (vector engine)
                    nc.vector.tensor_scalar(rl2[:, 512 * n:512 * (n + 1)],
                                            o2[:], b2s[:], 0.0,
                                            op0=ALU.add, op1=ALU.max)
                del rl1s[t]

            def back_fc1(t):
                # fc1 (+pool2, +bn3 scale): K=256 (2 chunks), M=16
                rl2 = rl2s[t]
                f1 = psf1.tile([16, 512], F32, tag="f1")
                for h in range(2):
                    nc.tensor.matmul(
                        f1[:],
                        w3s[:, 16 * h:16 * h + 16],
                        rl2[:, 512 * h:512 * h + 512],
                        start=(h == 0), stop=(h == 1),
                    )
                rl3 = relu3p.tile([16, 512], F32R, tag="rl3")
                nc.vector.tensor_scalar(rl3[:], f1[:], b3s[:], 0.0,
                                        op0=ALU.add, op1=ALU.max)
                del rl2s[t]
                return rl3

            def back_fc2(t, rl3):
                # fc2: K=16, M=8
                f2 = psf2.tile([8, 512], F32, tag="f2")
                nc.tensor.matmul(f2[:], w4s[:], rl3[:],
                                 start=True, stop=True)
                # expm1 = exp(x + fc2_b) - 1
                ob = outp.tile([8, 512], F32, tag="ob")
                nc.scalar.activation(ob[:], f2[:], AF.Exp, bias=b4s[:],
                                     scale=1.0)
                ob2 = outp.tile([8, 512], F32, tag="ob2")
                nc.vector.tensor_scalar(ob2[:], ob[:], 1.0, None,
                                        op0=ALU.subtract)
                nc.sync.dma_start(out_d[:, 4 * t:4 * t + 4, :], ob2[:])

            dma_in(0)
            nc.scalar.dma_start(ws[:], wtot_d[:])
            nc.scalar.dma_start(bs[:], btot_d[:])
            for s in range(NT + 2):
                t0, t1, t2 = s, s - 1, s - 2
                if s + 1 < NT:
                    dma_in(s + 1)
                rl3 = None
                if 0 <= t1 < NT:
                    mid(t1)
                if 0 <= t2 < NT:
                    rl3 = back_fc1(t2)
                if t0 < NT:
                    front(t0)
                if 0 <= t2 < NT:
                    back_fc2(t2, rl3)

    nc.compile()
    return nc


_NC = None


def _get_nc():
    global _NC
    if _NC is None:
        _NC = build_nc()
    return _NC


def _assemble(results):
    out = np.empty((4, 8, 128, 128), np.float32)
    for core in range(NCORES):
        b, half = divmod(core, 2)
        out[b, :, half * 64:half * 64 + 64, :] = results[core]["out"]
    return out


def kernel(_trace=False, **inputs):
    nc = _get_nc()
    in_maps = _make_in_maps(inputs)
    res = bass_utils.run_bass_kernel_spmd(
        nc, in_maps, core_ids=list(range(NCORES)), trace=_trace)
    out = _assemble(res.results)
    if _trace:
        return out, res
    return out


def kernel_sim(**inputs):
    """CoreSim-based check of a single core (core 0) against its slice."""
    from concourse.bass_interp import CoreSim
    nc = _get_nc()
    in_maps = _make_in_maps(inputs)
    outs = []
    for core in range(NCORES):
        sim = CoreSim(nc, trace=False, require_finite=False, require_nnan=False)
        for k, v in in_maps[core].items():
            sim.tensor(k)[:] = v
        sim.simulate()
        outs.append({"out": sim.tensor("out").copy()})
    return _assemble(outs)
